# revision 20
# baseline (speedup 1.0000x reference)
"""Local (sliding-window) attention kernel for TRN2, 8 NeuronCores.

Problem: B=32, N=8192, D=64 fp32; WINDOW=128, look_backward=1, look_forward=0,
pad_value=-1.0, softmax over the 256 (prev+own window) keys, no masking.

Sharding: batch rows 32 -> 8 cores x 4 rows (pure data parallel, no comms).

Per-core pipeline (per batch row, 64 windows):
  - DMA q/k/v tiles to SBUF (natural [token, d] layout).
  - PE transposes q (duplicated into both partition halves) and k (pairs) to
    D-major layout; DVE copies PSUM->SBUF round to float32r.
  - sim^T = k_tile @ q^T via one float32r matmul per key tile (N=256 covers
    both windows that see this key tile); even/odd tiles run in different
    PE row groups.
  - exp via ScalarE (scale=1/8 folded in) over 4-key-tile groups.
  - out = attn @ [v|1] via fp32 matmuls accumulating over the 2 key chunks;
    column 64 accumulates the softmax denominator.
  - DVE reciprocal + broadcast multiply normalizes; DMA store.
"""

import os
import numpy as np
from contextlib import ExitStack

import concourse.bass as bass
import concourse.tile as tile
from concourse import bacc, mybir
from concourse.bass_utils import run_bass_kernel_spmd
from concourse.masks import make_identity

F32 = mybir.dt.float32
F32R = mybir.dt.float32r
F16 = mybir.dt.float16
BF16 = mybir.dt.float16
EXP = mybir.ActivationFunctionType.Exp

B, N, D = 32, 8192, 64
W = 128                 # window size (tokens per tile)
NT = N // W             # 64 key/query tiles per batch row
NB = B // 8             # 4 batch rows per core
SCALE = D ** -0.5       # 0.125
PAD = -1.0

USE_F32R_SIM = os.environ.get("F32R", "1") == "1"  # tf32-rate QK^T matmul (input rounding ~2^-11)


def _emit(tc, ctx, q, k, v, o, nb=NB, nt=NT):
    nc = tc.nc
    NBL, NTL = nb, nt

    consts = ctx.enter_context(tc.tile_pool(name="consts", bufs=1))
    ident = consts.tile([128, 128], F16, name="ident")
    make_identity(nc, ident)

    # kT pad tile: -1.0 (dims x keys)
    kT_pad = consts.tile([64, 128], F16, name="kT_pad")
    nc.vector.memset(kT_pad, PAD)

    # [v|1] pad tile
    v_pad = consts.tile([128, 65], BF16, name="v_pad")
    nc.vector.memset(v_pad[:, 0:64], PAD)
    nc.vector.memset(v_pad[:, 64:65], 1.0)

    sim_dt = F16

    stage_pool = ctx.enter_context(tc.tile_pool(name="stage", bufs=2))
    qt2_pool = ctx.enter_context(tc.tile_pool(name="qt2", bufs=2))
    kt2_pool = ctx.enter_context(tc.tile_pool(name="kt2", bufs=2))
    v_pool = ctx.enter_context(tc.tile_pool(name="vreg", bufs=2))
    e_pool = ctx.enter_context(tc.tile_pool(name="epool", bufs=int(os.environ.get("EB","6"))))
    out_pool = ctx.enter_context(tc.tile_pool(name="outp", bufs=4))
    r_pool = ctx.enter_context(tc.tile_pool(name="rpool", bufs=4))

    tp_ps = ctx.enter_context(tc.tile_pool(name="tp_ps", bufs=2, space="PSUM"))
    sim_ps = ctx.enter_context(tc.tile_pool(name="sim_ps", bufs=int(os.environ.get("SIMB","2")), space="PSUM"))
    av_ps = ctx.enter_context(tc.tile_pool(name="av_ps", bufs=int(os.environ.get("AVB","2")), space="PSUM"))

    for b in range(NBL):
        qb = q[b].rearrange("(t p) d -> p t d", p=W)   # [128, 64, 64]
        kb = k[b].rearrange("(t p) d -> p t d", p=W)
        vb = v[b].rearrange("(t p) d -> p t d", p=W)
        ob = o[b].rearrange("(t p) d -> p t d", p=W)

        # whole-row v load (fp32) -> bf16 [v|1] region, ones in col 64
        v_f32 = stage_pool.tile([128, NTL, D], F32, name="v_f32", tag="vf", bufs=1)
        nc.sync.dma_start(v_f32, vb)
        v_sb = v_pool.tile([128, NTL, 65], BF16, name="v_sb")
        nc.vector.tensor_copy(v_sb[:, :, 0:64], v_f32)
        nc.vector.memset(v_sb[:, :, 64], 1.0)

        # ---- transposes to D-major: qT [64, NT*128], kT [64, NT*128]
        qT2 = qt2_pool.tile([64, NTL * W], sim_dt, name="qT2")
        kT2 = kt2_pool.tile([64, NTL * W], sim_dt, name="kT2")

        # chunked load -> cast -> transpose so the PE starts early
        NCH = max(1, NTL // 16)          # tiles per chunk = 16
        TCH = NTL // NCH
        for c in range(NCH):
            for dst, srcb, nm in ((qT2, qb, "q"), (kT2, kb, "k")):
                x_st = stage_pool.tile([128, TCH, D], F32, name="x_st", tag="xst" + nm)
                nc.gpsimd.dma_start(x_st, srcb[:, TCH * c:TCH * (c + 1)])
                x16 = stage_pool.tile([128, TCH, D], F16, name="x16", tag="x16" + nm)
                nc.vector.tensor_copy(x16, x_st)
                for jg in range(TCH // 8):
                    ps_t = tp_ps.tile([64, 1024], F16, name="ps_t", tag="tp")
                    for cc in range(8):
                        tl = 8 * jg + cc
                        nc.tensor.transpose(ps_t[:, 128 * cc:128 * (cc + 1)],
                                            x16[:, tl], ident)
                    g0 = TCH * c + 8 * jg
                    nc.vector.tensor_copy(dst[:, 128 * g0:128 * (g0 + 8)], ps_t)

        STAGE = int(os.environ.get("STAGE", "3"))
        if STAGE == 1:
            for g in range(NTL // 4):
                og1 = out_pool.tile([128, 4, 64], F32, name="og1", tag="og")
                nc.vector.tensor_copy(og1.rearrange("p a b -> p (a b)"),
                                      qT2[:, 256 * g:256 * (g + 1)].bitcast(F32))
                nc.sync.dma_start(ob[:, 4 * g:4 * (g + 1)], og1)
            continue

        # ---- sim^T + exp, over 65 slots (slot s <-> key tile t=s-1)
        # slot s cols [0:128] = sim^T(k_t, q_t) "own", [128:256] = sim^T(k_t, q_{t+1}) "prev"
        e_groups = []
        sim_groups = []
        ngroups = (NTL + 1 + 3) // 4
        for g in range(ngroups):
            nslots = min(4, NTL + 1 - 4 * g)
            sg = sim_ps.tile([128, 256 * nslots], F32, name="sg", tag="simg")
            eg = e_pool.tile([128, 256 * nslots], BF16, name="eg", tag="eg")
            sim_groups.append(sg)
            e_groups.append(eg)
            if g == 0:
                # slot 0 own-half is unused; define it for exp
                nc.vector.memset(sg[:, 0:128], 0.0)
            for c in range(nslots):
                s = 4 * g + c
                t = s - 1
                if t < 0:
                    lhsT = kT_pad
                    rhs = qT2[:, 0:W]
                    out = sg[:, 256 * c + 128:256 * c + 256]
                else:
                    lhsT = kT2[:, W * t:W * (t + 1)]
                    hi = min(t + 2, NTL)
                    rhs = qT2[:, W * t:W * hi]
                    out = sg[:, 256 * c:256 * c + W * (hi - t)]
                nc.tensor.matmul(out, lhsT, rhs, start=True, stop=True)
            # last slot has no "prev" half (no window NTL) -- don't exp it
            lim = 256 * nslots if 4 * g + nslots - 1 < NTL else 256 * nslots - 128
            if os.environ.get("NOEXP", "0") == "1":
                nc.vector.tensor_copy(eg[:, 0:lim], sg[:, 0:lim])
            else:
                nc.scalar.activation(eg[:, 0:lim], sg[:, 0:lim], EXP, scale=SCALE)

        if STAGE == 2:
            for g in range(NTL // 4):
                og2 = out_pool.tile([128, 4, 64], F32, name="og2", tag="og")
                nc.vector.tensor_copy(og2.rearrange("p a b -> p (a b)"),
                                      e_groups[g][:, 0:256])
                nc.sync.dma_start(ob[:, 4 * g:4 * (g + 1)], og2)
            continue

        def e_own(w):   # chunk [keys_w, q_w]
            s = w + 1
            return e_groups[s // 4][:, 256 * (s % 4):256 * (s % 4) + 128]

        def e_prev(w):  # chunk [keys_{w-1}, q_w]
            s = w
            return e_groups[s // 4][:, 256 * (s % 4) + 128:256 * (s % 4) + 256]

        # ---- AV + normalize, groups of 4 windows; store per 16 windows
        og = None
        for g in range(NTL // 4):
            if g % 4 == 0:
                og = out_pool.tile([128, min(16, NTL - 4 * g), 64], F32,
                                   name="og", tag="og")
            ag = av_ps.tile([128, 260], F32, name="ag")
            for c in range(4):
                w = 4 * g + c
                vprev = v_pad if w == 0 else v_sb[:, w - 1]
                nc.tensor.matmul(ag[:, 65 * c:65 * (c + 1)], e_prev(w), vprev,
                                 start=True, stop=False)
                nc.tensor.matmul(ag[:, 65 * c:65 * (c + 1)], e_own(w), v_sb[:, w],
                                 start=False, stop=True)
            agv = ag.rearrange("p (w c) -> p w c", c=65)
            r4 = r_pool.tile([128, 4], F32, name="r4")
            nc.vector.reciprocal(r4, agv[:, :, 64])
            nc.vector.tensor_mul(og[:, 4 * (g % 4):4 * (g % 4) + 4], agv[:, :, 0:64],
                                 r4.unsqueeze(2).broadcast_to((128, 4, 64)))
            if g % 4 == 3 or g == NTL // 4 - 1:
                g0 = 4 * (g // 4)
                nc.sync.dma_start(ob[:, 4 * g0:4 * (g + 1)], og)


_CACHED_NC = None


def _build():
    global _CACHED_NC
    if _CACHED_NC is not None:
        return _CACHED_NC
    nc = bacc.Bacc("TRN2", target_bir_lowering=False, debug=False, num_devices=8)
    q = nc.dram_tensor("q", [NB, N, D], F32, kind="ExternalInput").ap()
    k = nc.dram_tensor("k", [NB, N, D], F32, kind="ExternalInput").ap()
    v = nc.dram_tensor("v", [NB, N, D], F32, kind="ExternalInput").ap()
    o = nc.dram_tensor("o", [NB, N, D], F32, kind="ExternalOutput").ap()
    with tile.TileContext(nc) as tc, ExitStack() as ctx:
        _emit(tc, ctx, q, k, v, o)
    nc.compile()
    _CACHED_NC = nc
    return nc


def kernel(q, k, v, **run_kwargs):
    q = np.ascontiguousarray(q, dtype=np.float32)
    k = np.ascontiguousarray(k, dtype=np.float32)
    v = np.ascontiguousarray(v, dtype=np.float32)
    nc = _build()
    in_maps = [
        {"q": q[NB * c:NB * (c + 1)], "k": k[NB * c:NB * (c + 1)],
         "v": v[NB * c:NB * (c + 1)]}
        for c in range(8)
    ]
    res = run_bass_kernel_spmd(nc, in_maps, core_ids=list(range(8)), **run_kwargs)
    out = np.concatenate([res.results[c]["o"] for c in range(8)], axis=0)
    if run_kwargs.get("trace"):
        kernel.last_results = res
    return out
